# revision 1
# baseline (speedup 1.0000x reference)
"""2-layer dense GCN on 8 Trainium2 NeuronCores.

Reference computation (all fp32):
    H0 = relu((A_norm @ X) @ W0)
    H1 = relu((A_norm @ H0) @ W1)
A_norm: [16384, 16384], X: [16384, 128], W0/W1: [128, 128].

Sharding: 1D row partition of A_norm (2048 rows/core). Each core holds
A[rows_c].T (host-transposed so the node-contraction dim lands on SBUF
partitions), computes its row block of each layer, and the hidden state
is exchanged between layers with chunked on-device AllGathers.

Device layout is transpose-free:
  - aggregate:  psum[d, i] += X_tile[j, d].T @ A_T_tile[j, i]
                (lhsT = stationary node-major X/H tile, rhs = A^T slice)
  - linear:     psum[i, e]  = M^T_tile[d, i].T @ W[d, e]   (node-major out)
  - relu fused into the PSUM->SBUF eviction on the scalar engine.

The aggregation runs CHUNK-MAJOR (one 512-wide output chunk at a time,
full contraction each): chunk k's hidden tiles finish at ~(k+1)/4 of the
layer, so AllGather k overlaps the remaining chunks' compute — only the
last AllGather is exposed at the layer boundary. The stationary H layout
in SBUF ([128, 512] pieces) is exactly what the chunked AllGathers
produce, so no transposes are needed anywhere.

PRECISION modes:
  - "fp32":   exact fp32 matmuls (4 cyc/row on the PE).
  - "split3": A and X/H split into bf16 hi+lo; aggregate computed as
              Ah@Xh + Al@Xh + Ah@Xl (3 bf16 passes, ~2.5e-6 rel err —
              fp32-class).
  - "bf16":   plain bf16 aggregate (1 cyc/row, half the DMA bytes,
              ~1.1e-3 rel err).
"""

import sys
from contextlib import ExitStack

if "/opt/trn_rl_repo" not in sys.path:
    sys.path.insert(0, "/opt/trn_rl_repo")

import numpy as np

N_NODES = 16384
D = 128
NCORES = 8
ROWS = N_NODES // NCORES  # 2048

PRECISION = "bf16"  # "fp32" | "split3" | "bf16"


def _geom(n_nodes=N_NODES, ncores=NCORES, precision=PRECISION):
    esz = 4 if precision == "fp32" else 2
    nsplit = 2 if precision == "split3" else 1  # hi/lo operand copies
    rows = n_nodes // ncores
    jt = n_nodes // 128          # total j-tiles (contraction tiles)
    jt_per_rank = jt // ncores   # j-tiles covered by one rank's nodes
    ic = min(512, rows)          # i-chunk width (one PSUM bank, fp32 out)
    nch = rows // ic             # i-chunks per core
    # j-tiles per A DMA: ~2 MiB per transfer; deep buffer pool so the
    # A-stream prefetch (~16 MiB) covers the inter-layer AllGather window
    target = 2 * 1024 * 1024
    jg = max(1, target // (128 * rows * esz))
    jg = min(jg, jt)
    while jt % jg:
        jg -= 1
    a_bufs = {"bf16": 8, "fp32": 6, "split3": 3}[precision]
    return dict(
        esz=esz, nsplit=nsplit, rows=rows, jt=jt, jt_per_rank=jt_per_rank,
        ic=ic, nch=nch, jg=jg, ndma=jt // jg, a_bufs=a_bufs,
    )


def build_gcn(n_nodes=N_NODES, d=D, ncores=NCORES, precision=PRECISION):
    """Build the SPMD Bass program (one program, runs on all cores)."""
    import concourse.bass as bass  # noqa: F401
    import concourse.tile as tile
    from concourse import bacc, mybir

    F32 = mybir.dt.float32
    BF16 = mybir.dt.bfloat16
    agg_dt = F32 if precision == "fp32" else BF16

    g_ = _geom(n_nodes, ncores, precision)
    nsplit, rows, jt = g_["nsplit"], g_["rows"], g_["jt"]
    jt_per_rank, ic, nch = g_["jt_per_rank"], g_["ic"], g_["nch"]
    jg, ndma, a_bufs = g_["jg"], g_["ndma"], g_["a_bufs"]
    lt = ic // 128               # linear i-tiles (and h tiles) per chunk

    nc = bacc.Bacc("TRN2", target_bir_lowering=False, num_devices=ncores)

    # A^T shards, host pre-tiled: DMA group g is the contiguous block
    # a_in[g*128 : (g+1)*128, :], covering j-tiles [g*jg, (g+1)*jg) x
    # all output columns, with a_in[g*128+p, t*rows+i] = A^T[(g*jg+t)*128+p, i]
    a_in = [
        nc.dram_tensor(
            f"a{s}", [ndma * 128, jg * rows], agg_dt, kind="ExternalInput"
        )
        for s in range(nsplit)
    ]
    # x_t: X pre-tiled on host into the AllGather layout:
    # x_t[r*128 + p, tl*128 + dd] = X[(r*jt_per_rank + tl)*128 + p, dd]
    x_in = [
        nc.dram_tensor(f"x{s}", [ncores * 128, rows], agg_dt, kind="ExternalInput")
        for s in range(nsplit)
    ]
    w0 = nc.dram_tensor("w0", [d, d], F32, kind="ExternalInput")
    w1 = nc.dram_tensor("w1", [d, d], F32, kind="ExternalInput")
    h_out = nc.dram_tensor("h_out", [rows, d], F32, kind="ExternalOutput")

    relu = mybir.ActivationFunctionType.Relu

    with tile.TileContext(nc) as tc, ExitStack() as ctx:
        sb1 = ctx.enter_context(tc.tile_pool(name="sb1", bufs=1))
        stat_pool = ctx.enter_context(
            tc.tile_pool(name="stat", bufs=ncores * nsplit)
        )
        a_pool = ctx.enter_context(tc.tile_pool(name="a", bufs=a_bufs))
        m_pool = ctx.enter_context(tc.tile_pool(name="m", bufs=2))
        h_pool = ctx.enter_context(tc.tile_pool(name="h", bufs=4))
        split_pool = ctx.enter_context(tc.tile_pool(name="spl", bufs=4))
        agg_pool = ctx.enter_context(tc.tile_pool(name="agg", bufs=4, space="PSUM"))
        lin_pool = ctx.enter_context(tc.tile_pool(name="lin", bufs=2, space="PSUM"))
        dram = ctx.enter_context(tc.tile_pool(name="dram", bufs=1, space="DRAM"))

        w0_sb = sb1.tile([d, d], F32)
        nc.scalar.dma_start(out=w0_sb[:], in_=w0[:])
        w1_sb = sb1.tile([d, d], F32)
        nc.scalar.dma_start(out=w1_sb[:], in_=w1[:])

        def load_stat_chunks(srcs, lname):
            """srcs: per split s: [ncores*128, rows] DRAM view.
            Returns stat[s][r] = [128, rows] SBUF tile."""
            out = []
            for s in range(nsplit):
                chunks = []
                for r in range(ncores):
                    sc = stat_pool.tile(
                        [128, rows], agg_dt, name=f"{lname}{s}_{r}", tag="sc"
                    )
                    nc.gpsimd.dma_start(
                        out=sc[:], in_=srcs[s][r * 128 : (r + 1) * 128, :]
                    )
                    chunks.append(sc)
                out.append(chunks)
            return out

        def layer(stat, w_sb, write_out, layer_done):
            # stat[s][r]: stationary chunks; j-tile j lives in chunk
            # r=j//jt_per_rank at cols (j%jt_per_rank)*128
            passes = [(0, 0)] if nsplit == 1 else [(0, 0), (1, 0), (0, 1)]
            agg = [
                agg_pool.tile([128, ic], F32, name=f"ps{c}", tag="ps")
                for c in range(nch)
            ]
            for g in range(ndma):
                ats = []
                for s in range(nsplit):
                    at = a_pool.tile(
                        [128, jg * rows], agg_dt, name=f"at{s}", tag=f"at{s}"
                    )
                    eng = nc.sync if (g + s) % 2 == 0 else nc.scalar
                    eng.dma_start(
                        out=at[:], in_=a_in[s][g * 128 : (g + 1) * 128, :]
                    )
                    ats.append(at)
                for t in range(jg):
                    j = g * jg + t
                    jr = j % jt_per_rank
                    for pi, (ls, rs) in enumerate(passes):
                        lhs = stat[ls][j // jt_per_rank][
                            :, jr * 128 : (jr + 1) * 128
                        ]
                        for c in range(nch):
                            nc.tensor.matmul(
                                agg[c][:],
                                lhsT=lhs,
                                rhs=ats[rs][
                                    :, t * rows + c * ic : t * rows + (c + 1) * ic
                                ],
                                start=(j == 0 and pi == 0),
                                stop=(j == jt - 1 and pi == len(passes) - 1),
                            )
            # linear + relu, node-major output tiles
            for c in range(nch):
                mt = m_pool.tile([128, ic], F32, name="mt", tag="mt")
                nc.vector.tensor_copy(out=mt[:], in_=agg[c][:])
                for it in range(lt):
                    lp = lin_pool.tile([128, d], F32, name="lp", tag="lp")
                    nc.tensor.matmul(
                        lp[:],
                        lhsT=mt[:, it * 128 : (it + 1) * 128],
                        rhs=w_sb[:],
                        start=True,
                        stop=True,
                    )
                    ht = h_pool.tile([128, d], F32, name="ht", tag="ht")
                    nc.scalar.activation(ht[:], lp[:], relu)
                    write_out(c, it, ht)
            layer_done()

        # ---- layer 0 ----
        stat0 = load_stat_chunks([x[:] for x in x_in], "sx")
        # packed hidden-state bounce ([hi | lo] along free dim when split)
        h_tb = dram.tile([128, nsplit * rows], agg_dt, name="h_tb")
        h_ag = dram.tile(
            [ncores * 128, nsplit * rows], agg_dt, addr_space="Shared", name="h_ag"
        )

        def write_l0(c, it, ht):
            tl = c * lt + it
            if precision == "fp32":
                nc.scalar.dma_start(
                    out=h_tb[:, tl * 128 : (tl + 1) * 128], in_=ht[:]
                )
                return
            hh = split_pool.tile([128, d], BF16, name="hh", tag="hh")
            nc.vector.tensor_copy(out=hh[:], in_=ht[:])
            nc.scalar.dma_start(out=h_tb[:, tl * 128 : (tl + 1) * 128], in_=hh[:])
            if nsplit == 2:
                hh32 = split_pool.tile([128, d], F32, name="hh32", tag="hh32")
                nc.vector.tensor_copy(out=hh32[:], in_=hh[:])
                hl = split_pool.tile([128, d], BF16, name="hl", tag="hl")
                nc.vector.tensor_sub(out=hl[:], in0=ht[:], in1=hh32[:])
                nc.scalar.dma_start(
                    out=h_tb[:, rows + tl * 128 : rows + (tl + 1) * 128], in_=hl[:]
                )

        def ag_l0():
            import concourse.mybir as _mb

            nc.gpsimd.collective_compute(
                "AllGather",
                _mb.AluOpType.bypass,
                replica_groups=[list(range(ncores))],
                ins=[h_tb[:]],
                outs=[h_ag[:]],
            )

        layer(stat0, w0_sb, write_l0, ag_l0)

        # ---- layer 1 ----
        stat1 = load_stat_chunks(
            [h_ag[:, s * rows : (s + 1) * rows] for s in range(nsplit)], "sh"
        )

        def write_l1(c, it, ht):
            nc.scalar.dma_start(
                out=h_out[c * ic + it * 128 : c * ic + (it + 1) * 128, :], in_=ht[:]
            )

        layer(stat1, w1_sb, write_l1, lambda: None)

    nc.finalize()
    return nc


def _tile_stat(X, ncores, jt_per_rank):
    rows = jt_per_rank * 128
    return np.ascontiguousarray(
        X.reshape(ncores, jt_per_rank, 128, D).transpose(0, 2, 1, 3)
        .reshape(ncores * 128, rows)
    )


def shard_inputs(A_norm, X, n_nodes=N_NODES, ncores=NCORES, precision=PRECISION):
    """Host-side shard prep. Returns per-core input maps."""
    import ml_dtypes

    bf16 = ml_dtypes.bfloat16
    g_ = _geom(n_nodes, ncores, precision)
    rows, jt_per_rank = g_["rows"], g_["jt_per_rank"]
    jg, ndma = g_["jg"], g_["ndma"]

    def tile_a(a_tc):
        # [n_nodes, rows] -> [ndma*128, jg*rows] so DMA group g is the
        # contiguous block a_pre[g*128:(g+1)*128, :] with
        # a_pre[g*128+p, t*rows+i] = a_tc[(g*jg+t)*128+p, i]
        return np.ascontiguousarray(
            a_tc.reshape(ndma, jg, 128, rows).swapaxes(1, 2)
            .reshape(ndma * 128, jg * rows)
        )

    x_t = _tile_stat(X, ncores, jt_per_rank)
    if precision == "fp32":
        xs = [x_t]
    else:
        x_hi = x_t.astype(bf16)
        xs = [x_hi]
        if precision == "split3":
            xs.append((x_t - x_hi.astype(np.float32)).astype(bf16))

    in_maps = []
    for c in range(ncores):
        a_tc = np.ascontiguousarray(A_norm[c * rows : (c + 1) * rows, :].T)
        m = {}
        if precision == "fp32":
            m["a0"] = tile_a(a_tc)
        else:
            a_hi = a_tc.astype(bf16)
            m["a0"] = tile_a(a_hi)
            if precision == "split3":
                m["a1"] = tile_a((a_tc - a_hi.astype(np.float32)).astype(bf16))
        for s, x in enumerate(xs):
            m[f"x{s}"] = x
        in_maps.append(m)
    return in_maps


_CACHED = {}


def kernel(A_norm, X, W0, W1):
    A_norm = np.ascontiguousarray(A_norm, dtype=np.float32)
    X = np.ascontiguousarray(X, dtype=np.float32)
    W0 = np.ascontiguousarray(W0, dtype=np.float32)
    W1 = np.ascontiguousarray(W1, dtype=np.float32)

    from concourse.bass_utils import run_bass_kernel_spmd

    if PRECISION not in _CACHED:
        _CACHED[PRECISION] = build_gcn(precision=PRECISION)
    nc = _CACHED[PRECISION]

    in_maps = shard_inputs(A_norm, X, precision=PRECISION)
    for m in in_maps:
        m["w0"] = W0
        m["w1"] = W1

    res = run_bass_kernel_spmd(nc, in_maps, core_ids=list(range(NCORES)))
    return np.concatenate([res.results[c]["h_out"] for c in range(NCORES)], axis=0)



# revision 4
# speedup vs baseline: 1.4733x; 1.4733x over previous
"""2-layer dense GCN on 8 Trainium2 NeuronCores — fp8 residual edition.

Reference computation (all fp32):
    H0 = relu((A_norm @ X) @ W0)
    H1 = relu((A_norm @ H0) @ W1)
A_norm: [16384, 16384] row-stochastic, X: [16384, 128], W0/W1: [128, 128].

Key idea: A_norm rows sum to exactly 1, so A = (1/N)*ones + R with R
zero-mean uniform in [-1/N, ~1/N]. The rank-1 part is computed exactly as a
per-feature bias (mu * colsum(H)); only the residual R is streamed, encoded
as fp8 e3m4 (4 mantissa bits, narrow range — ideal for the uniform residual).
This halves HBM traffic vs bf16 at equal accuracy (~1.2e-3 rel err).

Device layout (per core, 2048 output rows, 1D row shard):
  - aggregation: psum[d, i] += stat[j, d].T @ Rq^T[j, i]   (lhsT = bf16
    stationary X/H tile, rhs = e3m4 residual slice — mixed-dtype matmul,
    1 col/cycle). Chunk-major: one 512-row output chunk per full-contraction
    pass, so chunk c's hidden tiles finish at (c+1)/4 of the layer.
  - rank-1 bias: sigma'[d] = sum_j stat[j, d] * (s*mu) via width-2
    ones-matmuls (bf16 hi/lo constant) accumulated over the contraction of
    chunk 0; layer-0 bias comes precomputed from the host. Applied at PSUM
    eviction (vector tensor_scalar_add), output in bf16.
  - linear: lp[i, e] = mt[d, i].T @ (W/s)[d, e] in bf16, relu on scalar
    engine, fp32 result.
  - inter-layer exchange: 4 chunked AllGathers (one per 512-row chunk),
    issued as each chunk's H tiles complete -> overlap with the remaining
    chunks' compute; layer 1 consumes stationary chunks in quarter-major
    order so it can start as soon as AllGather 0 lands.

A-stream: 16 blocks x 2 MiB per layer ((chunk, quarter) granularity,
contiguous), alternating on the two HWDGE rings (sync/scalar). Stationary
loads ride SWDGE (gpsimd/vector) so AllGather-gated loads never head-of-line
block the A stream.
"""

import sys
from contextlib import ExitStack

if "/opt/trn_rl_repo" not in sys.path:
    sys.path.insert(0, "/opt/trn_rl_repo")

import numpy as np

N_NODES = 16384
D = 128
NCORES = 8
ROWS = N_NODES // NCORES      # 2048
NCH = 4                       # output chunks per core (512 rows each)
IC = ROWS // NCH              # 512
NQ = 4                        # quarters of the per-peer contraction
NT = 4                        # j-subtiles per (quarter, peer)

PRECISION = "e3m4"  # tag for test.py compatibility


def build_gcn():
    import concourse.bass as bass  # noqa: F401
    import concourse.tile as tile
    from concourse import bacc, mybir

    F32 = mybir.dt.float32
    BF16 = mybir.dt.bfloat16
    E3 = mybir.dt.float8e3
    relu = mybir.ActivationFunctionType.Relu
    add = mybir.AluOpType.add

    nc = bacc.Bacc("TRN2", target_bir_lowering=False, num_devices=NCORES)

    # A residual, host pre-tiled: block (c, q) is rows [(c*4+q)*128, +128),
    # all 16384 cols; element (p, (r*4+t)*512 + cc) =
    #   s * R[myrows0 + c*512 + cc, ((r*16 + q*4 + t)*128 + p)]
    a_in = nc.dram_tensor(
        "a0", [NCH * NQ * 128, NCORES * NT * IC], E3, kind="ExternalInput"
    )
    # X stationary quarter tiles: row (q*8+r)*128+p, col t*128+dd =
    #   X[r*2048 + q*512 + t*128 + p, dd] in bf16
    x_in = nc.dram_tensor("x0", [NQ * NCORES * 128, IC], BF16, kind="ExternalInput")
    w0 = nc.dram_tensor("w0", [D, D], BF16, kind="ExternalInput")  # W0 / s
    w1 = nc.dram_tensor("w1", [D, D], BF16, kind="ExternalInput")  # W1 / s
    b0 = nc.dram_tensor("b0", [D, 1], F32, kind="ExternalInput")   # s*mu*colsum(X)
    c1 = nc.dram_tensor("c1", [D, 2], BF16, kind="ExternalInput")  # [hi, lo] of s*mu
    h_out = nc.dram_tensor("h_out", [ROWS, D], F32, kind="ExternalOutput")

    with tile.TileContext(nc) as tc, ExitStack() as ctx:
        sb1 = ctx.enter_context(tc.tile_pool(name="sb1", bufs=1))
        stat_pool = ctx.enter_context(tc.tile_pool(name="stat", bufs=2 * NQ * NCORES))
        a_pool = ctx.enter_context(tc.tile_pool(name="a", bufs=5))
        m_pool = ctx.enter_context(tc.tile_pool(name="m", bufs=2))
        h_pool = ctx.enter_context(tc.tile_pool(name="h", bufs=4))
        agg_pool = ctx.enter_context(tc.tile_pool(name="agg", bufs=2, space="PSUM"))
        sig_pool = ctx.enter_context(tc.tile_pool(name="sig", bufs=1, space="PSUM"))
        lin_pool = ctx.enter_context(tc.tile_pool(name="lin", bufs=2, space="PSUM"))
        dram = ctx.enter_context(tc.tile_pool(name="dram", bufs=1, space="DRAM"))

        w0_sb = sb1.tile([D, D], BF16)
        nc.sync.dma_start(out=w0_sb[:], in_=w0[:])
        w1_sb = sb1.tile([D, D], BF16)
        nc.scalar.dma_start(out=w1_sb[:], in_=w1[:])
        b0_sb = sb1.tile([D, 1], F32)
        nc.sync.dma_start(out=b0_sb[:], in_=b0[:])
        c1_sb = sb1.tile([D, 2], BF16)
        nc.scalar.dma_start(out=c1_sb[:], in_=c1[:])

        # layer-0 stationary (X): HWDGE up front, before the A stream
        stat0 = []
        for q in range(NQ):
            row = []
            for r in range(NCORES):
                t_ = stat_pool.tile([128, IC], BF16, name=f"sx{q}_{r}", tag="sc")
                eng = nc.sync if (q * NCORES + r) % 2 == 0 else nc.scalar
                eng.dma_start(
                    out=t_[:], in_=x_in[(q * NCORES + r) * 128 : (q * NCORES + r + 1) * 128, :]
                )
                row.append(t_)
            stat0.append(row)

        h_tb = [dram.tile([128, IC], BF16, name=f"h_tb{c}") for c in range(NCH)]
        h_ag = [
            dram.tile([NCORES * 128, IC], BF16, addr_space="Shared", name=f"h_ag{c}")
            for c in range(NCH)
        ]

        def layer(stat, w_sb, is_l1):
            sig_sb = None
            for c in range(NCH):
                agg = agg_pool.tile([128, IC], F32, name="ps", tag="ps")
                if is_l1 and c == 0:
                    sig = sig_pool.tile([128, 2], F32, name="sg", tag="sg")
                for q in range(NQ):
                    blk = c * NQ + q
                    at = a_pool.tile(
                        [128, NCORES * NT * IC], E3, name="at", tag="at"
                    )
                    eng = nc.sync if blk % 2 == 0 else nc.scalar
                    eng.dma_start(out=at[:], in_=a_in[blk * 128 : (blk + 1) * 128, :])
                    for r in range(NCORES):
                        for t in range(NT):
                            first = q == 0 and r == 0 and t == 0
                            last = q == NQ - 1 and r == NCORES - 1 and t == NT - 1
                            lhsT = stat[q][r][:, t * 128 : (t + 1) * 128]
                            nc.tensor.matmul(
                                agg[:],
                                lhsT=lhsT,
                                rhs=at[:, (r * NT + t) * IC : (r * NT + t + 1) * IC],
                                start=first,
                                stop=last,
                            )
                            if is_l1 and c == 0:
                                nc.tensor.matmul(
                                    sig[:],
                                    lhsT=lhsT,
                                    rhs=c1_sb[:],
                                    start=first,
                                    stop=last,
                                )
                # bias: layer 0 from host; layer 1 from the sigma matmuls
                if is_l1 and c == 0:
                    sig2_sb = sb1.tile([D, 2], F32, name="sig2_sb")
                    nc.vector.tensor_copy(out=sig2_sb[:], in_=sig[:])
                    sig_sb = sb1.tile([D, 1], F32, name="sig_sb")
                    nc.vector.tensor_tensor(
                        out=sig_sb[:], in0=sig2_sb[:, 0:1], in1=sig2_sb[:, 1:2], op=add
                    )
                bias = sig_sb if is_l1 else b0_sb
                mt = m_pool.tile([128, IC], BF16, name="mt", tag="mt")
                nc.vector.tensor_scalar_add(out=mt[:], in0=agg[:], scalar1=bias[:])
                for t in range(NT):
                    lp = lin_pool.tile([128, D], F32, name="lp", tag="lp")
                    nc.tensor.matmul(
                        lp[:],
                        lhsT=mt[:, t * 128 : (t + 1) * 128],
                        rhs=w_sb[:],
                        start=True,
                        stop=True,
                    )
                    ht = h_pool.tile([128, D], F32, name="ht", tag="ht")
                    nc.scalar.activation(ht[:], lp[:], relu)
                    if is_l1:
                        nc.gpsimd.dma_start(
                            out=h_out[c * IC + t * 128 : c * IC + (t + 1) * 128, :],
                            in_=ht[:],
                        )
                    else:
                        hh = h_pool.tile([128, D], BF16, name="hh", tag="hh")
                        nc.vector.tensor_copy(out=hh[:], in_=ht[:])
                        nc.gpsimd.dma_start(
                            out=h_tb[c][:, t * 128 : (t + 1) * 128], in_=hh[:]
                        )
                if not is_l1:
                    import concourse.mybir as _mb

                    nc.gpsimd.collective_compute(
                        "AllGather",
                        _mb.AluOpType.bypass,
                        replica_groups=[list(range(NCORES))],
                        ins=[h_tb[c][:]],
                        outs=[h_ag[c][:]],
                    )

        layer(stat0, w0_sb, is_l1=False)

        # layer-1 stationary from the chunked AllGathers (SWDGE; each load
        # only waits on its own AllGather)
        stat1 = []
        for q in range(NQ):
            row = []
            for r in range(NCORES):
                t_ = stat_pool.tile([128, IC], BF16, name=f"sh{q}_{r}", tag="sc")
                nc.gpsimd.dma_start(out=t_[:], in_=h_ag[q][r * 128 : (r + 1) * 128, :])
                row.append(t_)
            stat1.append(row)

        layer(stat1, w1_sb, is_l1=True)

    nc.finalize()
    return nc


def shard_inputs(A_norm, X, W0, W1, precision=None):
    """Host-side prep. Returns per-core input maps (complete, incl. weights)."""
    import ml_dtypes

    bf16 = ml_dtypes.bfloat16
    e3m4 = ml_dtypes.float8_e3m4
    N = N_NODES
    mu = np.float32(1.0 / N)

    R = A_norm.astype(np.float32) - mu
    s = float(ml_dtypes.finfo(e3m4).max) / float(np.abs(R).max())
    Rq = (R * np.float32(s)).astype(e3m4)
    del R

    Xb = X.astype(bf16)
    # stationary quarter tiles: [q, r, p, t, d] -> [(q*8+r)*128+p, t*128+d]
    x0 = np.ascontiguousarray(
        Xb.reshape(NCORES, NQ, NT, 128, D)
        .transpose(1, 0, 3, 2, 4)
        .reshape(NQ * NCORES * 128, IC)
    )

    smu = np.float32(s * mu)
    hi = bf16(smu)
    lo = bf16(np.float32(smu - np.float32(hi)))
    c1 = np.broadcast_to(np.array([hi, lo], dtype=bf16), (D, 2)).copy()
    b0 = (
        np.float64(s) * np.float64(mu) * Xb.astype(np.float64).sum(axis=0)
    ).astype(np.float32).reshape(D, 1)
    w0 = (W0.astype(np.float64) / s).astype(bf16)
    w1 = (W1.astype(np.float64) / s).astype(bf16)

    in_maps = []
    for core in range(NCORES):
        Rt = Rq[core * ROWS : (core + 1) * ROWS, :].T  # [16384 nodes, 2048]
        a0 = np.ascontiguousarray(
            Rt.reshape(NCORES, NQ, NT, 128, NCH, IC)
            .transpose(4, 1, 3, 0, 2, 5)
            .reshape(NCH * NQ * 128, NCORES * NT * IC)
        )
        in_maps.append(
            {"a0": a0, "x0": x0, "w0": w0, "w1": w1, "b0": b0, "c1": c1}
        )
    return in_maps


_CACHED = {}


def kernel(A_norm, X, W0, W1):
    A_norm = np.ascontiguousarray(A_norm, dtype=np.float32)
    X = np.ascontiguousarray(X, dtype=np.float32)
    W0 = np.ascontiguousarray(W0, dtype=np.float32)
    W1 = np.ascontiguousarray(W1, dtype=np.float32)

    from concourse.bass_utils import run_bass_kernel_spmd

    if PRECISION not in _CACHED:
        _CACHED[PRECISION] = build_gcn()
    nc = _CACHED[PRECISION]

    in_maps = shard_inputs(A_norm, X, W0, W1)
    res = run_bass_kernel_spmd(nc, in_maps, core_ids=list(range(NCORES)))
    return np.concatenate([res.results[c]["h_out"] for c in range(NCORES)], axis=0)
